# revision 39
# baseline (speedup 1.0000x reference)
"""JointNetwork Trainium2 kernel.

out[b,t,u,f] = (audio[b] @ W[:H])[t,f] + (label[b] @ W[H:])[u,f] + bias[f]

Sharding: data-parallel over B — B=8 batch elements map 1:1 onto the 8
NeuronCores; no communication.

Per-core plan (memory regime). The output is written to HBM as bf16
(rel-err cost ~2^-9, far inside the 2e-2 gate) and upcast to fp32 on the
host — halving the dominant HBM write from 64 MiB to 32 MiB per core.

  1. bf16 inputs; PE transposes build audioT/labelT; bf16 matmuls
     project a = audio@Wa and l = label@Wl + b into fp32 PSUM.  The
     projections land in four combined SBUF tiles ral[c][s] [128, F]:
     partitions 0-63 hold a-rows 64s..64s+63 of t-chunk c, partitions
     64-127 hold l (all 64 u-rows, bias folded in).
  2. Streams 128 output tiles [128 rows, F], rows = 2 t-values x 64
     u-values.  ONE one-hot stationary matrix per tile (two 1s per
     column: the a-row and the l-row) makes each N=512 matmul compute
     a[t]+l[u] directly, so a tile costs exactly 2 matmuls and its
     drain is a pure fp32->bf16 copy.
  3. Drains alternate DVE / ACT copies (both capped at 1x by the fp32
     PSUM source).  Tiles are grouped x4 into [128, 4F] SBUF buffers so
     each out-DMA moves 1 MiB; DMAs alternate sync (HWDGE) and gpsimd
     (SWDGE) queues.  The host un-permutes the group layout and upcasts.
"""

import numpy as np

B, T, U, H, F = 8, 256, 64, 512, 1024
N_CORES = 8
NTILES = (T * U) // 128  # 128 output tiles of [128, F] per core
TPC = T // 128  # t-chunks
KC = H // 128  # contraction chunks for projections

GROUP = 4  # output tiles per SBUF buffer / DMA (1 MiB per transfer)
OUT_BUFS = 8  # [128, GROUP*F] bf16 group buffers
PSUM_BUFS = 4  # [128, F] fp32 tiles (2 banks each)


def _is_act_tile(i):
    # drain split: ~47% of tiles ACT copy, rest DVE copy
    return i % 15 in (1, 3, 5, 7, 9, 11, 13)


def _build_nc():
    import concourse.bacc as bacc
    import concourse.mybir as mybir
    import concourse.tile as tile

    f32 = mybir.dt.float32
    bf16 = mybir.dt.bfloat16

    nc = bacc.Bacc("TRN2", target_bir_lowering=False, debug=False)

    audio_d = nc.dram_tensor("audio", [T, H], bf16, kind="ExternalInput")
    label_d = nc.dram_tensor("label", [U, H], bf16, kind="ExternalInput")
    w_d = nc.dram_tensor("w", [2 * H, F], bf16, kind="ExternalInput")
    bias_d = nc.dram_tensor("bias", [1, F], bf16, kind="ExternalInput")
    selc_d = nc.dram_tensor("selc", [128, 64 * 128], bf16, kind="ExternalInput")
    ident_d = nc.dram_tensor("ident", [128, 128], bf16, kind="ExternalInput")
    ones1_d = nc.dram_tensor("ones1", [1, 128], bf16, kind="ExternalInput")
    # group layout: group g holds tiles 4g..4g+3 as [128, 4F]; the host
    # un-permutes rows (g, m, h, f) -> row 128*(4g+h)+m afterwards
    out_d = nc.dram_tensor(
        "out", [NTILES // GROUP, 128, GROUP * F], bf16, kind="ExternalOutput"
    )

    with tile.TileContext(nc) as tc:
        with (
            tc.tile_pool(name="const", bufs=1) as cpool,
            tc.tile_pool(name="w", bufs=1) as wpool,
            tc.tile_pool(name="proj", bufs=1) as ppool,
            tc.tile_pool(name="psum", bufs=PSUM_BUFS, space="PSUM") as ps_pool,
            tc.tile_pool(name="out", bufs=OUT_BUFS) as opool,
        ):
            # ---- consts + raw inputs first so PE transposes start early ----
            ident = cpool.tile([128, 128], bf16)
            nc.sync.dma_start(out=ident[:], in_=ident_d[:])
            label_sb = ppool.tile([U, H], bf16, tag="label")
            nc.scalar.dma_start(out=label_sb[:], in_=label_d[:])
            audio_sb = []
            for c in range(TPC):
                at = ppool.tile([128, H], bf16, tag=f"audio{c}", name=f"audio{c}")
                nc.sync.dma_start(out=at[:], in_=audio_d[c * 128 : (c + 1) * 128, :])
                audio_sb.append(at)
            ones1 = cpool.tile([1, 128], bf16)
            nc.scalar.dma_start(out=ones1[:], in_=ones1_d[:])
            bias = cpool.tile([1, F], bf16)
            nc.scalar.dma_start(out=bias[:], in_=bias_d[:])
            selc = cpool.tile([128, 64 * 128], bf16)
            nc.gpsimd.dma_start(out=selc[:], in_=selc_d[:])

            # ---- weights: wl half (feeds l) on scalar ring; wa on sync ----
            wtiles = [None] * (2 * KC)
            for k in range(KC, 2 * KC):
                wt = wpool.tile([128, F], bf16, tag=f"w{k}", name=f"w{k}")
                nc.scalar.dma_start(out=wt[:], in_=w_d[k * 128 : (k + 1) * 128, :])
                wtiles[k] = wt
            for k in range(KC):
                wt = wpool.tile([128, F], bf16, tag=f"w{k}", name=f"w{k}")
                nc.sync.dma_start(out=wt[:], in_=w_d[k * 128 : (k + 1) * 128, :])
                wtiles[k] = wt

            # ---- PE transposes: audioT[k]/labelT2[k] = in[:, k*128:+128].T
            # label_t2 duplicates the 64 l-columns twice -> M=128 l-proj
            # directly yields [l; l] across all 128 PSUM partitions.
            audio_t = [
                ppool.tile([128, T], bf16, tag=f"at{k}", name=f"at{k}")
                for k in range(KC)
            ]
            label_t2 = [
                ppool.tile([128, 128], bf16, tag=f"lt{k}", name=f"lt{k}")
                for k in range(KC)
            ]
            for k in range(KC):
                pt = ps_pool.tile([128, 2 * F], bf16, tag="ps", name="pt")
                nc.tensor.transpose(
                    pt[:, 0:U], label_sb[:, k * 128 : (k + 1) * 128], ident[0:U, 0:U]
                )
                nc.vector.tensor_copy(out=label_t2[k][:, 0:U], in_=pt[:, 0:U])
                nc.vector.tensor_copy(out=label_t2[k][:, U:128], in_=pt[:, 0:U])
                for c in range(TPC):
                    pt = ps_pool.tile([128, 2 * F], bf16, tag="ps", name="pt")
                    nc.tensor.transpose(
                        pt[:, 0:128], audio_sb[c][:, k * 128 : (k + 1) * 128], ident[:]
                    )
                    nc.vector.tensor_copy(
                        out=audio_t[k][:, c * 128 : (c + 1) * 128], in_=pt[:, 0:128]
                    )

            # ---- combined tiles:
            # ral[c][0] = [a rows 0..63   (p 0..63)  ; l (p 64..127)]
            # ral[c][1] = [l (p 0..63)   ; a rows 64..127 (p 64..127)]
            # so every projection->ral copy stays on its own partitions.
            ral = [
                [
                    ppool.tile([128, F], bf16, tag=f"ral{c}{s}", name=f"ral{c}{s}")
                    for s in range(2)
                ]
                for c in range(TPC)
            ]

            # l projection -> [l; l] on all 128 PSUM partitions (bias folded)
            pl2 = ps_pool.tile([128, F], f32, tag="ps", name="pl2")
            for nh in range(2):
                sl = slice(nh * 512, (nh + 1) * 512)
                for k in range(KC):
                    nc.tensor.matmul(
                        pl2[:, sl],
                        lhsT=label_t2[k][:, :],
                        rhs=wtiles[KC + k][:, sl],
                        start=(k == 0),
                        stop=False,
                    )
                nc.tensor.matmul(
                    pl2[:, sl],
                    lhsT=ones1[:, :],
                    rhs=bias[:, sl],
                    start=False,
                    stop=True,
                )
            nc.scalar.copy(out=ral[0][0][64:128, :], in_=pl2[64:128, :])
            nc.vector.tensor_copy(out=ral[0][1][0:64, :], in_=pl2[0:64, :])
            nc.scalar.copy(out=ral[1][0][64:128, :], in_=pl2[64:128, :])
            nc.vector.tensor_copy(out=ral[1][1][0:64, :], in_=pl2[0:64, :])

            # a projection (M=128) -> halves copied to matching partitions
            for c in range(TPC):
                pa = ps_pool.tile([128, F], f32, tag="ps", name="pa")
                for nh in range(2):
                    sl = slice(nh * 512, (nh + 1) * 512)
                    for k in range(KC):
                        nc.tensor.matmul(
                            pa[:, sl],
                            lhsT=audio_t[k][:, c * 128 : (c + 1) * 128],
                            rhs=wtiles[k][:, sl],
                            start=(k == 0),
                            stop=(k == KC - 1),
                        )
                nc.scalar.copy(out=ral[c][0][0:64, :], in_=pa[0:64, :])
                nc.vector.tensor_copy(out=ral[c][1][64:128, :], in_=pa[64:128, :])

            # ---- stream: groups of GROUP [128, F] tiles ----
            for g in range(NTILES // GROUP):
                ot = opool.tile([128, GROUP * F], bf16)
                for h in range(GROUP):
                    i = GROUP * g + h
                    c, j = divmod(i, 64)
                    s = j // 32
                    po = ps_pool.tile([128, F], f32, tag="ps", name="po")
                    for nh in range(2):
                        sl = slice(nh * 512, (nh + 1) * 512)
                        nc.tensor.matmul(
                            po[:, sl],
                            lhsT=selc[:, j * 128 : (j + 1) * 128],
                            rhs=ral[c][s][:, sl],
                            start=True,
                            stop=True,
                        )
                    osl = slice(h * F, (h + 1) * F)
                    if _is_act_tile(i):
                        nc.scalar.copy(out=ot[:, osl], in_=po[:])
                    else:
                        nc.vector.tensor_copy(out=ot[:, osl], in_=po[:])
                eng = (nc.sync, nc.gpsimd, nc.scalar)[g % 3]
                eng.dma_start(out=out_d[g], in_=ot[:])

    nc.compile()
    return nc


_NC = None


def _get_nc():
    global _NC
    if _NC is None:
        _NC = _build_nc()
    return _NC


def _host_consts():
    import ml_dtypes

    bf = ml_dtypes.bfloat16
    # selc[k, 128j + m]: two ones per column (a-row + l-row); the a/l
    # partition halves swap between s = j//32 = 0 and 1 (ral layout)
    selc = np.zeros((128, 64 * 128), dtype=bf)
    for j in range(64):
        a_base = 0 if j < 32 else 64
        l_base = 64 if j < 32 else 0
        ja = a_base + 2 * (j % 32)
        selc[ja, 128 * j : 128 * j + 64] = 1.0
        selc[ja + 1, 128 * j + 64 : 128 * j + 128] = 1.0
        for m in range(128):
            selc[l_base + m % 64, 128 * j + m] = 1.0
    ones1 = np.ones((1, 128), dtype=bf)
    ident = np.eye(128, dtype=bf)
    return selc, ones1, ident


def _in_maps(audio_vector, label_vector, W, b):
    import ml_dtypes

    bf = ml_dtypes.bfloat16
    selc, ones1, ident = _host_consts()
    wb = np.ascontiguousarray(W).astype(bf)
    maps = []
    for i in range(N_CORES):
        maps.append(
            {
                "audio": np.ascontiguousarray(audio_vector[i]).astype(bf),
                "label": np.ascontiguousarray(label_vector[i]).astype(bf),
                "w": wb,
                "bias": np.ascontiguousarray(b).astype(bf).reshape(1, F),
                "selc": selc,
                "ones1": ones1,
                "ident": ident,
            }
        )
    return maps


def _run(in_maps, **kw):
    from concourse.bass_utils import run_bass_kernel_spmd

    nc = _get_nc()
    return run_bass_kernel_spmd(nc, in_maps, core_ids=list(range(N_CORES)), **kw)


def _unpack(raw):
    # raw: [NTILES//GROUP, 128, GROUP*F] bf16, tile h of group g in cols
    # h*F:(h+1)*F -> row-major [T*U, F] with row 128*(GROUP*g+h)+m
    a = raw.astype(np.float32).reshape(NTILES // GROUP, 128, GROUP, F)
    return a.transpose(0, 2, 1, 3).reshape(T, U, F)


def kernel(audio_vector, label_vector, W, b):
    res = _run(_in_maps(audio_vector, label_vector, W, b))
    out = np.stack([_unpack(res.results[i]["out"]) for i in range(N_CORES)])
    return out
